# revision 6
# baseline (speedup 1.0000x reference)
"""MoE FFN (E=8 experts, top-2 routing, SwiGLU) on 8 TRN2 NeuronCores.

Strategy (expert-parallel, per sharding hint):
  - Host computes the tiny gate (x @ Wg, 0.07% of total FLOPs), top-2 routing
    and softmax combine weights. This IS the sharding step: tokens are
    dispatched (gathered) per expert, one expert per core.
  - Each core runs the SwiGLU FFN for its expert over its routed tokens in
    bf16 (fp32 accumulation in PSUM), scales rows by the combine weight.
  - Host scatter-adds the 8 per-expert outputs back into the full [T, D]
    output (the unshard step).

Device kernel layout (per core, capacity C tokens, padded with zeros; all
matmuls bf16 with fp32 PSUM accumulation, tokens always the moving dim so
cost scales with the real token count C_comp):
  phase 1:  hT[hid, tok] = silu(W1.T @ xT) * (W3.T @ xT)
            lhsT = W1/W3 tile [128d, 128h] (stationary), rhs = xT [128d, ntok]
  phase 2:  yT[d, tok] = W2.T @ hT, scaled elementwise by the combine weight
            (host-broadcast [128, C] tile) during PSUM eviction on DVE.

Ramp design: a single HWDGE DMA issue costs ~600ns of sequencer time
(Sync/Scalar are the only HWDGE engines) and a single DMA transfer moves
~90GB/s, so the first k-sweep can't have all its inputs for several us.
Chunk-0 phase 1 therefore runs k-OUTER with 4-wide hb groups (4 live PSUM
accumulation banks): the PE consumes exactly in DMA arrival order —
x[k] + w1[k] low-columns land k-by-k while the PE chews 4x512-col matmuls
per k.  W2 is host-relaid db-slab-major so each phase-2 sweep depends on
one DMA.
"""

import os
import sys

import numpy as np

for _p in ("/opt/trn_rl_repo",):
    if os.path.isdir(_p) and _p not in sys.path:
        sys.path.insert(0, _p)

import ml_dtypes

DIM = 1024
HID = 2048
E = 8
TOPK = 2
P = 128
NCORES = 8
TBS = 512  # moving-dim token chunk for phase 1

KD = DIM // P   # 8  k-chunks over DIM
KH = HID // P   # 16 k-chunks over HID
GW = 2          # hb-group width for chunk-0 k-outer sweeps (2 => groups are
                # double-buffered in the 4-deep ph1 PSUM rotation, so a
                # group's eviction chain never stalls the next group)

BF16 = ml_dtypes.bfloat16

_KERNEL_CACHE = {}
LAST_RESULT = None  # BassKernelResults of the most recent run (for test.py)


def _chunks_for(C_comp):
    """Moving-dim chunks covering the C_comp real tokens. Chunk 0 is a full
    512 so the W1/W3 k-outer passes stretch past the weight arrivals; the
    remainder splits evenly into chunks >~250 so LDWEIGHTS (97ns) stays
    hidden behind each matmul."""
    chunks = []
    remaining = C_comp
    while remaining > 768:
        chunks.append(TBS)
        remaining -= TBS
    if remaining > 512:
        h = remaining // 2
        chunks += [remaining - h, h]
    elif remaining:
        chunks.append(remaining)
    return chunks


def _build(C, C_comp):
    import concourse.mybir as mybir
    import concourse.tile as tile
    from concourse import bacc

    f32 = mybir.dt.float32
    bf16 = mybir.dt.bfloat16
    AF = mybir.ActivationFunctionType

    chunks = _chunks_for(C_comp)
    HH = HID // 4  # w1 low-column split covering the first hb group

    nc = bacc.Bacc(None, target_bir_lowering=False, debug=False)

    xT = nc.declare_dram_parameter("xT", [DIM, C], bf16, isOutput=False)
    w1 = nc.declare_dram_parameter("w1", [DIM, HID], bf16, isOutput=False)
    w3 = nc.declare_dram_parameter("w3", [DIM, HID], bf16, isOutput=False)
    # db-slab-major W2: row block db holds lhsT[k, db-block] for all 16
    # k-chunks contiguously, so one DMA feeds a whole phase-2 sweep.
    w2s = nc.declare_dram_parameter("w2s", [KD * P, KH * P], bf16, isOutput=False)
    wb = nc.declare_dram_parameter("wb", [P, C], f32, isOutput=False)
    out = nc.declare_dram_parameter("out", [DIM, C], bf16, isOutput=True)

    with tile.TileContext(nc) as tc:
        with (
            tc.tile_pool(name="persist", bufs=1) as const,
            tc.tile_pool(name="psA", bufs=3, space="PSUM") as psA,
            tc.tile_pool(name="psY", bufs=1, space="PSUM") as psY,
            tc.tile_pool(name="sil", bufs=3) as sil_pool,
            tc.tile_pool(name="ysb", bufs=2) as y_pool,
        ):
            xT_sb = [const.tile([P, C], bf16, name=f"xT{k}", tag=f"xT{k}") for k in range(KD)]
            w1_sb = [const.tile([P, HID], bf16, name=f"w1{k}", tag=f"w1{k}") for k in range(KD)]
            w3_sb = [const.tile([P, HID], bf16, name=f"w3{k}", tag=f"w3{k}") for k in range(KD)]
            w2_sb = [const.tile([P, KH * P], bf16, name=f"w2{d}", tag=f"w2{d}") for d in range(KD)]
            wb_sb = const.tile([P, C], f32, tag="wb")
            hT_sb = [const.tile([P, C], bf16, name=f"hT{k}", tag=f"hT{k}") for k in range(KH)]
            warm = const.tile([P, 256], bf16, tag="warm")
            nc.vector.memset(warm[:], 0.0)
            zb = const.tile([P, 1], f32, tag="zb")
            nc.vector.memset(zb[:], 0.0)

            # PE warmup: dummy matmuls until the first real operands land.
            # Keeping the PE continuously busy from ~1us into exec also
            # satisfies the HAM clock-ramp (full speed needs ~3.4us of
            # uninterrupted execution), so real matmuls run at 2.4GHz soon
            # after the first operands arrive.
            for _ in range(11):
                wp = psA.tile([P, TBS], f32, tag="ph1", bufs=4, name="warmp")
                nc.tensor.matmul(wp[:, :256], lhsT=warm[:, :P], rhs=warm[:])

            # ── input DMA schedule ────────────────────────────────────────
            # Two HWDGE issuers (~600ns/issue); in-flight DMAs complete in
            # near-bulk order (SDMA engines round-robin all queues at packet
            # granularity), so what matters is that the ramp-critical bytes
            # are SMALL and ISSUED FIRST in consumption order.  Chunk-0's
            # first half-sweep needs only x[k] cols 0:256 (0.5 MB) plus the
            # w1[k] low columns (1 MB), so x chunk-0 is split into two
            # 256-col ladders: the PE starts real work ~2us earlier and the
            # half-B ladder lands while half-A is being consumed.
            c0 = min(chunks[0], C)
            ch = c0 // 2
            # Scalar: x chunk-0 halves then x high cols — the Scalar engine
            # is clear of issue work before the eviction sigmoids start
            # (~18us), or the ph1 PSUM rotation would stall the PE.
            for k in range(KD):
                nc.scalar.dma_start(out=xT_sb[k][:, :ch], in_=xT[k * P:(k + 1) * P, :ch])
            for k in range(KD):
                nc.scalar.dma_start(out=xT_sb[k][:, ch:c0], in_=xT[k * P:(k + 1) * P, ch:c0])
            if c0 < C:
                for k in range(KD):
                    nc.scalar.dma_start(out=xT_sb[k][:, c0:], in_=xT[k * P:(k + 1) * P, c0:])

            def sy(out_ap, in_ap):
                nc.sync.dma_start(out=out_ap, in_=in_ap)

            # Sync: w1 in 512-col quarter-tranches consumption-ordered, then
            # w3 halves, wb, w2 slabs (in need order).
            QW = 512
            for q in range(HID // QW):
                for k in range(KD):
                    sy(w1_sb[k][:, q * QW:(q + 1) * QW], w1[k * P:(k + 1) * P, q * QW:(q + 1) * QW])
            for k in range(KD):
                sy(w3_sb[k][:, :HID // 2], w3[k * P:(k + 1) * P, :HID // 2])
            for k in range(KD):
                sy(w3_sb[k][:, HID // 2:], w3[k * P:(k + 1) * P, HID // 2:])
            sy(wb_sb[:], wb[:, :])
            for d in range(KD):
                sy(w2_sb[d][:], w2s[d * P:(d + 1) * P, :])

            # ── phase 1, chunk 0: k-outer, hb groups of GW ───────────────
            # Two passes (W1 then W3) so chunk-0 compute only depends on W1
            # + x at the start; silu(h1) staged as bf16 in slu.
            n0 = chunks[0]
            slu = [
                const.tile([P, n0], bf16, name=f"slu{hb}", tag=f"slu{hb}")
                for hb in range(KH)
            ]

            # Each group-pair sweep is column-split into two half-sweeps so
            # the first real matmuls depend on only half the x chunk-0
            # bytes.  Each [*, a:b] half accumulates independently in PSUM
            # (has_written bits are per element); the eviction reads the
            # full [*, :n0] once both halves' stops retire.
            def ko_pass(w_sb, evict, split_first):
                for g in range(KH // GW):
                    phs = [
                        psA.tile([P, TBS], f32, tag="ph1", bufs=4, name=f"ph1g{j}")
                        for j in range(GW)
                    ]
                    halves = ((0, ch), (ch, n0)) if (split_first and g == 0) else ((0, n0),)
                    for a, b in halves:
                        for k in range(KD):
                            for j in range(GW):
                                hb = g * GW + j
                                nc.tensor.matmul(
                                    phs[j][:, a:b],
                                    lhsT=w_sb[k][:, hb * P:(hb + 1) * P],
                                    rhs=xT_sb[k][:, a:b],
                                    start=(k == 0),
                                    stop=(k == KD - 1),
                                )
                    for j in range(GW):
                        evict(g * GW + j, phs[j])

            def evict_w1(hb, ph):
                sil = sil_pool.tile([P, TBS], f32, tag="sil")
                nc.scalar.activation(sil[:, :n0], ph[:, :n0], AF.Sigmoid, bias=zb[:])
                nc.vector.tensor_mul(slu[hb][:], sil[:, :n0], ph[:, :n0])

            def evict_w3(hb, ph):
                nc.vector.tensor_mul(hT_sb[hb][:, :n0], slu[hb][:], ph[:, :n0])

            ko_pass(w1_sb, evict_w1, split_first=True)
            ko_pass(w3_sb, evict_w3, split_first=False)

            # ── phase 1, remaining chunks: fused per-hid-block ph1/ph3 ───
            def mm_sweep(dst_psum, w_sb, hb, t0, n):
                for k in range(KD):
                    nc.tensor.matmul(
                        dst_psum[:, :n],
                        lhsT=w_sb[k][:, hb * P:(hb + 1) * P],
                        rhs=xT_sb[k][:, t0:t0 + n],
                        start=(k == 0),
                        stop=(k == KD - 1),
                    )

            t0 = n0
            for n in chunks[1:]:
                for hb in range(KH):
                    ph1 = psA.tile([P, TBS], f32, tag="ph1", bufs=4)
                    ph3 = psA.tile([P, TBS], f32, tag="ph3", bufs=2)
                    mm_sweep(ph1, w1_sb, hb, t0, n)
                    mm_sweep(ph3, w3_sb, hb, t0, n)
                    # silu(h1)*h3 = sigmoid(h1)*h1*h3 (Silu isn't in CoreSim)
                    sil = sil_pool.tile([P, TBS], f32, tag="sil")
                    sg2 = sil_pool.tile([P, TBS], f32, tag="sg2")
                    nc.scalar.activation(sil[:, :n], ph1[:, :n], AF.Sigmoid, bias=zb[:])
                    nc.vector.tensor_mul(sg2[:, :n], sil[:, :n], ph1[:, :n])
                    nc.vector.tensor_mul(
                        hT_sb[hb][:, t0:t0 + n], sg2[:, :n], ph3[:, :n]
                    )
                t0 += n

            # ── phase 2: yT[d, tok] = W2.T @ h ───────────────────────────
            # tokens as the moving dim so cost scales with real tokens;
            # combine weight applied elementwise against a host-broadcast
            # [P, C] tile during PSUM eviction (DVE).
            t0 = 0
            chunks2 = chunks
            for ci, n in enumerate(chunks2):
                last_chunk = ci == len(chunks2) - 1
                for db in range(KD):
                    py = psY.tile([P, TBS], f32, tag="py", bufs=2)
                    for k in range(KH):
                        nc.tensor.matmul(
                            py[:, :n],
                            lhsT=w2_sb[db][:, k * P:(k + 1) * P],
                            rhs=hT_sb[k][:, t0:t0 + n],
                            start=(k == 0),
                            stop=(k == KH - 1),
                        )
                    if last_chunk and db == KD - 1:
                        # tail: split the final eviction (small second piece)
                        # so the two DMAs overlap on the two HWDGE issuers.
                        ha = (3 * n) // 4
                        ysa = y_pool.tile([P, TBS], bf16, tag="y")
                        nc.vector.tensor_mul(
                            ysa[:, :ha], py[:, :ha], wb_sb[:, t0:t0 + ha]
                        )
                        nc.scalar.dma_start(
                            out=out[db * P:(db + 1) * P, t0:t0 + ha],
                            in_=ysa[:, :ha],
                        )
                        ysb2 = y_pool.tile([P, TBS], bf16, tag="y")
                        nc.vector.tensor_mul(
                            ysb2[:, :n - ha], py[:, ha:n], wb_sb[:, t0 + ha:t0 + n]
                        )
                        nc.sync.dma_start(
                            out=out[db * P:(db + 1) * P, t0 + ha:t0 + n],
                            in_=ysb2[:, :n - ha],
                        )
                    else:
                        ysb = y_pool.tile([P, TBS], bf16, tag="y")
                        nc.vector.tensor_mul(ysb[:, :n], py[:, :n], wb_sb[:, t0:t0 + n])
                        # alternate the out-DMA issuer (Scalar is idle in
                        # phase 2) so neither HWDGE stream becomes the
                        # eviction bottleneck.
                        eng = nc.sync if db % 2 == 0 else nc.scalar
                        eng.dma_start(
                            out=out[db * P:(db + 1) * P, t0:t0 + n], in_=ysb[:, :n]
                        )
                t0 += n

    nc.compile()
    return nc


def _get_kernel(C, C_comp):
    key = (C, C_comp)
    nc = _KERNEL_CACHE.get(key)
    if nc is None:
        nc = _build(C, C_comp)
        _KERNEL_CACHE[key] = nc
    return nc


def _route(xt, Wg):
    """Host gate: returns per-expert (token_indices, combine_weights)."""
    scores = xt.astype(np.float32) @ Wg.astype(np.float32)          # [T, E]
    top2 = np.argpartition(-scores, 1, axis=1)[:, :2]               # [T, 2]
    vals = np.take_along_axis(scores, top2, axis=1)                 # [T, 2]
    vals = vals - vals.max(axis=1, keepdims=True)
    ev = np.exp(vals)
    sm = ev / ev.sum(axis=1, keepdims=True)                         # [T, 2]
    T = xt.shape[0]
    combine = np.zeros((T, E), dtype=np.float32)
    combine[np.arange(T)[:, None], top2] = sm
    idx = []
    wts = []
    for e in range(E):
        ie = np.nonzero(combine[:, e])[0]
        idx.append(ie)
        wts.append(combine[ie, e])
    return idx, wts


def _slab_w2(w):
    """[HID, DIM] -> [KD*P, KH*P] db-slab-major bf16: row block db holds, at
    [p, k*P + c], the element w[k*P + p, db*P + c]."""
    v = np.asarray(w, dtype=np.float32).reshape(KH, P, KD, P)
    return np.ascontiguousarray(v.transpose(2, 1, 0, 3).reshape(KD * P, KH * P)).astype(BF16)


def kernel(x, Wg, W1, W3, W2):
    global LAST_RESULT
    from concourse import bass_utils

    orig_shape = x.shape
    orig_dtype = x.dtype
    xt = np.ascontiguousarray(np.asarray(x, dtype=np.float32).reshape(-1, DIM))
    T = xt.shape[0]

    idx, wts = _route(xt, np.asarray(Wg, dtype=np.float32))
    max_n = max(len(i) for i in idx)
    C = max(P, -(-max_n // P) * P)
    C_comp = max(1, max_n)

    nc = _get_kernel(C, C_comp)

    W1 = np.asarray(W1)
    W3 = np.asarray(W3)
    W2 = np.asarray(W2)
    in_maps = []
    for e in range(E):
        n_e = len(idx[e])
        xT_e = np.zeros((DIM, C), dtype=BF16)
        xT_e[:, :n_e] = np.ascontiguousarray(xt[idx[e]].T).astype(BF16)
        wv_pad = np.zeros(C, dtype=np.float32)
        wv_pad[:n_e] = wts[e]
        wb_e = np.ascontiguousarray(np.broadcast_to(wv_pad, (P, C)))
        in_maps.append(
            {
                "xT": xT_e,
                "w1": W1[e].astype(BF16),
                "w3": W3[e].astype(BF16),
                "w2s": _slab_w2(W2[e]),
                "wb": wb_e,
            }
        )

    res = bass_utils.run_bass_kernel_spmd(nc, in_maps, core_ids=list(range(NCORES)))
    LAST_RESULT = res

    out = np.zeros((T, DIM), dtype=np.float32)
    for e in range(E):
        n_e = len(idx[e])
        if n_e:
            out[idx[e]] += np.asarray(
                res.results[e]["out"][:, :n_e], dtype=np.float32
            ).T
    return out.reshape(orig_shape).astype(orig_dtype, copy=False)



# revision 13
# speedup vs baseline: 1.0661x; 1.0661x over previous
"""MoE FFN (E=8 experts, top-2 routing, SwiGLU) on 8 TRN2 NeuronCores.

Strategy (expert-parallel, per sharding hint):
  - Host computes the tiny gate (x @ Wg, 0.07% of total FLOPs), top-2 routing
    and softmax combine weights. This IS the sharding step: tokens are
    dispatched (gathered) per expert, one expert per core.
  - Each core runs the SwiGLU FFN for its expert over its routed tokens in
    bf16 (fp32 accumulation in PSUM), scales rows by the combine weight.
  - Host scatter-adds the 8 per-expert outputs back into the full [T, D]
    output (the unshard step).

Device kernel layout (per core, capacity C tokens, padded with zeros; all
matmuls bf16 with fp32 PSUM accumulation, tokens always the moving dim so
cost scales with the real token count C_comp):
  phase 1:  hT[hid, tok] = silu(W1.T @ xT) * (W3.T @ xT)
            lhsT = W1/W3 tile [128d, 128h] (stationary), rhs = xT [128d, ntok]
  phase 2:  yT[d, tok] = W2.T @ hT, scaled elementwise by the combine weight
            (host-broadcast [128, C] tile) during PSUM eviction on DVE.

Ramp design: a single HWDGE DMA issue costs ~600ns of sequencer time
(Sync/Scalar are the only HWDGE engines) and a single DMA transfer moves
~90GB/s, so the first k-sweep can't have all its inputs for several us.
Chunk-0 phase 1 therefore runs k-OUTER with 4-wide hb groups (4 live PSUM
accumulation banks): the PE consumes exactly in DMA arrival order —
x[k] + w1[k] low-columns land k-by-k while the PE chews 4x512-col matmuls
per k.  W2 is host-relaid db-slab-major so each phase-2 sweep depends on
one DMA.
"""

import os
import sys

import numpy as np

for _p in ("/opt/trn_rl_repo",):
    if os.path.isdir(_p) and _p not in sys.path:
        sys.path.insert(0, _p)

import ml_dtypes

DIM = 1024
HID = 2048
E = 8
TOPK = 2
P = 128
NCORES = 8
TBS = 512  # moving-dim token chunk for phase 1

KD = DIM // P   # 8  k-chunks over DIM
KH = HID // P   # 16 k-chunks over HID
GW = 2          # hb-group width for chunk-0 k-outer sweeps (2 => groups are
                # double-buffered in the 4-deep ph1 PSUM rotation, so a
                # group's eviction chain never stalls the next group)

BF16 = ml_dtypes.bfloat16

_KERNEL_CACHE = {}
LAST_RESULT = None  # BassKernelResults of the most recent run (for test.py)


def _chunks_for(C_comp):
    """Moving-dim chunks covering the C_comp real tokens. Chunk 0 is a full
    512 so the W1/W3 k-outer passes stretch past the weight arrivals; the
    remainder splits evenly into chunks >~250 so LDWEIGHTS (97ns) stays
    hidden behind each matmul."""
    chunks = []
    remaining = C_comp
    while remaining > 768:
        chunks.append(TBS)
        remaining -= TBS
    if remaining > 512:
        h = remaining // 2
        chunks += [remaining - h, h]
    elif remaining:
        chunks.append(remaining)
    return chunks


def _build(C, C_comp):
    import concourse.mybir as mybir
    import concourse.tile as tile
    from concourse import bacc

    f32 = mybir.dt.float32
    bf16 = mybir.dt.bfloat16
    AF = mybir.ActivationFunctionType

    chunks = _chunks_for(C_comp)
    HH = HID // 4  # w1 low-column split covering the first hb group

    nc = bacc.Bacc(None, target_bir_lowering=False, debug=False)

    xT = nc.declare_dram_parameter("xT", [DIM, C], bf16, isOutput=False)
    w1 = nc.declare_dram_parameter("w1", [DIM, HID], bf16, isOutput=False)
    w3 = nc.declare_dram_parameter("w3", [DIM, HID], bf16, isOutput=False)
    # db-slab-major W2: row block db holds lhsT[k, db-block] for all 16
    # k-chunks contiguously, so one DMA feeds a whole phase-2 sweep.
    w2s = nc.declare_dram_parameter("w2s", [KD * P, KH * P], bf16, isOutput=False)
    wb = nc.declare_dram_parameter("wb", [P, C], f32, isOutput=False)
    out = nc.declare_dram_parameter("out", [DIM, C], bf16, isOutput=True)

    with tile.TileContext(nc) as tc:
        with (
            tc.tile_pool(name="persist", bufs=1) as const,
            tc.tile_pool(name="psA", bufs=3, space="PSUM") as psA,
            tc.tile_pool(name="psY", bufs=1, space="PSUM") as psY,
            tc.tile_pool(name="sil", bufs=3) as sil_pool,
            tc.tile_pool(name="ysb", bufs=2) as y_pool,
        ):
            xT_sb = [const.tile([P, C], bf16, name=f"xT{k}", tag=f"xT{k}") for k in range(KD)]
            w1_sb = [const.tile([P, HID], bf16, name=f"w1{k}", tag=f"w1{k}") for k in range(KD)]
            w3_sb = [const.tile([P, HID], bf16, name=f"w3{k}", tag=f"w3{k}") for k in range(KD)]
            w2_sb = [const.tile([P, KH * P], bf16, name=f"w2{d}", tag=f"w2{d}") for d in range(KD)]
            wb_sb = const.tile([P, C], f32, tag="wb")
            hT_sb = [const.tile([P, C], bf16, name=f"hT{k}", tag=f"hT{k}") for k in range(KH)]
            warm = const.tile([P, 256], bf16, tag="warm")
            nc.vector.memset(warm[:], 0.0)
            zb = const.tile([P, 1], f32, tag="zb")
            nc.vector.memset(zb[:], 0.0)

            # PE warmup: dummy matmuls until the first real operands land
            # (~11us: issue latency + ~128KB transfer on the saturated
            # queues). Keeping the PE continuously busy from ~7.8us also
            # satisfies the HAM clock-ramp (full speed needs ~3us of
            # uninterrupted execution), so real matmuls start at 2.4GHz.
            for _ in range(13):
                wp = psA.tile([P, TBS], f32, tag="ph1", bufs=4, name="warmp")
                nc.tensor.matmul(wp[:, :256], lhsT=warm[:, :P], rhs=warm[:])

            # ── input DMA schedule ────────────────────────────────────────
            # Two HWDGE issuers (~600ns/issue); the 16 DMA queues saturate at
            # ~333GB/s aggregate during the ramp, so what matters is that
            # bytes are queued in CONSUMPTION order at ~128-256KB granularity
            # (a transfer's completion gates every matmul touching it).
            # Chunk-0 k-outer consumption: step k of hb-group-pair q needs
            # x[k] chunk-0 and w1[k] cols [512q : 512(q+1)].
            c0 = min(chunks[0], C)
            # Scalar: ONLY x chunk-0 (the first pass's moving operand) — the
            # Scalar engine must be free for the eviction sigmoids from
            # ~13us on, or the ph1 PSUM rotation stalls the PE.
            for k in range(KD):
                nc.scalar.dma_start(out=xT_sb[k][:, :c0], in_=xT[k * P:(k + 1) * P, :c0])

            def sy(out_ap, in_ap):
                nc.sync.dma_start(out=out_ap, in_=in_ap)

            # Sync: w1 in 512-col quarter-tranches consumption-ordered, then
            # w3 halves, x high cols, wb, w2 slabs (in need order).
            QW = 512
            for q in range(HID // QW):
                for k in range(KD):
                    sy(w1_sb[k][:, q * QW:(q + 1) * QW], w1[k * P:(k + 1) * P, q * QW:(q + 1) * QW])
            for k in range(KD):
                sy(w3_sb[k][:, :HID // 2], w3[k * P:(k + 1) * P, :HID // 2])
            for k in range(KD):
                sy(w3_sb[k][:, HID // 2:], w3[k * P:(k + 1) * P, HID // 2:])
            if c0 < C:
                for k in range(KD):
                    sy(xT_sb[k][:, c0:], xT[k * P:(k + 1) * P, c0:])
            sy(wb_sb[:], wb[:, :])
            for d in range(KD):
                sy(w2_sb[d][:], w2s[d * P:(d + 1) * P, :])

            # ── phase 1, chunk 0: k-outer, hb groups of GW ───────────────
            # Two passes (W1 then W3) so chunk-0 compute only depends on W1
            # + x at the start; silu(h1) staged as bf16 in slu.
            n0 = chunks[0]
            slu = [
                const.tile([P, n0], bf16, name=f"slu{hb}", tag=f"slu{hb}")
                for hb in range(KH)
            ]

            def ko_pass(w_sb, evict):
                for g in range(KH // GW):
                    phs = [
                        psA.tile([P, TBS], f32, tag="ph1", bufs=4, name=f"ph1g{j}")
                        for j in range(GW)
                    ]
                    for k in range(KD):
                        for j in range(GW):
                            hb = g * GW + j
                            nc.tensor.matmul(
                                phs[j][:, :n0],
                                lhsT=w_sb[k][:, hb * P:(hb + 1) * P],
                                rhs=xT_sb[k][:, :n0],
                                start=(k == 0),
                                stop=(k == KD - 1),
                            )
                    for j in range(GW):
                        evict(g * GW + j, phs[j])

            def evict_w1(hb, ph):
                sil = sil_pool.tile([P, TBS], f32, tag="sil")
                nc.scalar.activation(sil[:, :n0], ph[:, :n0], AF.Sigmoid, bias=zb[:])
                nc.vector.tensor_mul(slu[hb][:], sil[:, :n0], ph[:, :n0])

            def evict_w3(hb, ph):
                nc.vector.tensor_mul(hT_sb[hb][:, :n0], slu[hb][:], ph[:, :n0])

            ko_pass(w1_sb, evict_w1)
            ko_pass(w3_sb, evict_w3)

            # ── phase 1, remaining chunks: fused per-hid-block ph1/ph3 ───
            def mm_sweep(dst_psum, w_sb, hb, t0, n):
                for k in range(KD):
                    nc.tensor.matmul(
                        dst_psum[:, :n],
                        lhsT=w_sb[k][:, hb * P:(hb + 1) * P],
                        rhs=xT_sb[k][:, t0:t0 + n],
                        start=(k == 0),
                        stop=(k == KD - 1),
                    )

            t0 = n0
            for n in chunks[1:]:
                for hb in range(KH):
                    ph1 = psA.tile([P, TBS], f32, tag="ph1", bufs=4)
                    ph3 = psA.tile([P, TBS], f32, tag="ph3", bufs=2)
                    mm_sweep(ph1, w1_sb, hb, t0, n)
                    mm_sweep(ph3, w3_sb, hb, t0, n)
                    # silu(h1)*h3 = sigmoid(h1)*h1*h3 (Silu isn't in CoreSim)
                    sil = sil_pool.tile([P, TBS], f32, tag="sil")
                    sg2 = sil_pool.tile([P, TBS], f32, tag="sg2")
                    nc.scalar.activation(sil[:, :n], ph1[:, :n], AF.Sigmoid, bias=zb[:])
                    nc.vector.tensor_mul(sg2[:, :n], sil[:, :n], ph1[:, :n])
                    nc.vector.tensor_mul(
                        hT_sb[hb][:, t0:t0 + n], sg2[:, :n], ph3[:, :n]
                    )
                t0 += n

            # ── phase 2: yT[d, tok] = W2.T @ h ───────────────────────────
            # tokens as the moving dim so cost scales with real tokens;
            # combine weight applied elementwise against a host-broadcast
            # [P, C] tile during PSUM eviction (DVE).
            t0 = 0
            chunks2 = chunks
            for ci, n in enumerate(chunks2):
                last_chunk = ci == len(chunks2) - 1
                for db in range(KD):
                    py = psY.tile([P, TBS], f32, tag="py", bufs=2)
                    for k in range(KH):
                        nc.tensor.matmul(
                            py[:, :n],
                            lhsT=w2_sb[db][:, k * P:(k + 1) * P],
                            rhs=hT_sb[k][:, t0:t0 + n],
                            start=(k == 0),
                            stop=(k == KH - 1),
                        )
                    if last_chunk and db == KD - 1:
                        # tail: split the final eviction (small second piece)
                        # so the two DMAs overlap on the two HWDGE issuers.
                        ha = (3 * n) // 4
                        ysa = y_pool.tile([P, TBS], bf16, tag="y")
                        nc.vector.tensor_mul(
                            ysa[:, :ha], py[:, :ha], wb_sb[:, t0:t0 + ha]
                        )
                        nc.scalar.dma_start(
                            out=out[db * P:(db + 1) * P, t0:t0 + ha],
                            in_=ysa[:, :ha],
                        )
                        ysb2 = y_pool.tile([P, TBS], bf16, tag="y")
                        nc.vector.tensor_mul(
                            ysb2[:, :n - ha], py[:, ha:n], wb_sb[:, t0 + ha:t0 + n]
                        )
                        nc.sync.dma_start(
                            out=out[db * P:(db + 1) * P, t0 + ha:t0 + n],
                            in_=ysb2[:, :n - ha],
                        )
                    else:
                        ysb = y_pool.tile([P, TBS], bf16, tag="y")
                        nc.vector.tensor_mul(ysb[:, :n], py[:, :n], wb_sb[:, t0:t0 + n])
                        # alternate the out-DMA issuer (Scalar is idle in
                        # phase 2) so neither HWDGE stream becomes the
                        # eviction bottleneck.
                        eng = nc.sync if db % 2 == 0 else nc.scalar
                        eng.dma_start(
                            out=out[db * P:(db + 1) * P, t0:t0 + n], in_=ysb[:, :n]
                        )
                t0 += n

    nc.compile()
    return nc


def _get_kernel(C, C_comp):
    key = (C, C_comp)
    nc = _KERNEL_CACHE.get(key)
    if nc is None:
        nc = _build(C, C_comp)
        _KERNEL_CACHE[key] = nc
    return nc


def _route(xt, Wg):
    """Host gate: returns per-expert (token_indices, combine_weights)."""
    scores = xt.astype(np.float32) @ Wg.astype(np.float32)          # [T, E]
    top2 = np.argpartition(-scores, 1, axis=1)[:, :2]               # [T, 2]
    vals = np.take_along_axis(scores, top2, axis=1)                 # [T, 2]
    vals = vals - vals.max(axis=1, keepdims=True)
    ev = np.exp(vals)
    sm = ev / ev.sum(axis=1, keepdims=True)                         # [T, 2]
    T = xt.shape[0]
    combine = np.zeros((T, E), dtype=np.float32)
    combine[np.arange(T)[:, None], top2] = sm
    idx = []
    wts = []
    for e in range(E):
        ie = np.nonzero(combine[:, e])[0]
        idx.append(ie)
        wts.append(combine[ie, e])
    return idx, wts


def _slab_w2(w):
    """[HID, DIM] -> [KD*P, KH*P] db-slab-major bf16: row block db holds, at
    [p, k*P + c], the element w[k*P + p, db*P + c]."""
    v = np.asarray(w, dtype=np.float32).reshape(KH, P, KD, P)
    return np.ascontiguousarray(v.transpose(2, 1, 0, 3).reshape(KD * P, KH * P)).astype(BF16)


def kernel(x, Wg, W1, W3, W2):
    global LAST_RESULT
    from concourse import bass_utils

    orig_shape = x.shape
    orig_dtype = x.dtype
    xt = np.ascontiguousarray(np.asarray(x, dtype=np.float32).reshape(-1, DIM))
    T = xt.shape[0]

    idx, wts = _route(xt, np.asarray(Wg, dtype=np.float32))
    max_n = max(len(i) for i in idx)
    C = max(P, -(-max_n // P) * P)
    C_comp = max(1, max_n)

    nc = _get_kernel(C, C_comp)

    W1 = np.asarray(W1)
    W3 = np.asarray(W3)
    W2 = np.asarray(W2)
    in_maps = []
    for e in range(E):
        n_e = len(idx[e])
        xT_e = np.zeros((DIM, C), dtype=BF16)
        xT_e[:, :n_e] = np.ascontiguousarray(xt[idx[e]].T).astype(BF16)
        wv_pad = np.zeros(C, dtype=np.float32)
        wv_pad[:n_e] = wts[e]
        wb_e = np.ascontiguousarray(np.broadcast_to(wv_pad, (P, C)))
        in_maps.append(
            {
                "xT": xT_e,
                "w1": W1[e].astype(BF16),
                "w3": W3[e].astype(BF16),
                "w2s": _slab_w2(W2[e]),
                "wb": wb_e,
            }
        )

    res = bass_utils.run_bass_kernel_spmd(nc, in_maps, core_ids=list(range(NCORES)))
    LAST_RESULT = res

    out = np.zeros((T, DIM), dtype=np.float32)
    for e in range(E):
        n_e = len(idx[e])
        if n_e:
            out[idx[e]] += np.asarray(
                res.results[e]["out"][:, :n_e], dtype=np.float32
            ).T
    return out.reshape(orig_shape).astype(orig_dtype, copy=False)

